# revision 42
# baseline (speedup 1.0000x reference)
"""CHESHIRE hyperedge link predictor on 8 Trainium2 NeuronCores.

Structure exploited (verified at runtime):
  - members[e] = base[e] + arange(8): each hyperedge is a contiguous
    8-node window -> sorting hyperedges by base makes the whole problem
    embarrassingly parallel across a node-range partition (no collectives).
  - edge_index is the full directed 8-clique per hyperedge -> deg == 7,
    w == -1/7, so Lhat(h) = (h - sum(h))/7 and the K=3 ChebConv folds into
    a single per-entry GEMM plus a per-window GEMM:
        u_i = (r * x_i) @ Wap + (r * S) @ Wd + D0
    with r = 1/sqrt(var+eps) (GraphNorm fold), S = window sum of x.
  - clip commutes with max/min pools; clip(u)^2 == min(u^2, 1) for the rms
    pool, so hardtanh is applied once to u before all three pools.

Per-core schedule (engine-balanced):
  encoder GEMM (fp16 in, f32 psum) -> DVE fused clip+move to xT (bf16)
  -> PE transpose -> ACT copy -> x rows to DRAM -> SWDGE transpose
  gathers -> per-chunk phase B emitted stats-one-chunk-ahead of
  gemm+pools so the DVE never waits on PE/ACT.  GraphNorm variance is
  computed CENTERED (ctr = x - alpha*mean, var = sum ctr^2) so bf16
  arithmetic cannot cancel catastrophically (the uncentered Q - k*S^2
  form goes NaN through sqrt for near-constant windows).  Per-entry
  GEMMs accumulate wap/wd into 4-bank PSUM tiles moved by single fused
  ACT identity+bias ops; one 4x-mode DVE tensor_scalar clips all 8
  entry planes; pools run as bf16 trees (DVE, ctr^2-tree level 1 on
  GPSIMD); sigmoid is deferred to one final ACT pass so the sqrt
  act-table stays loaded throughout (2 table loads total).
"""

import math
import os

import numpy as np

N_CORES = 8
M = 8          # nodes per hyperedge
D = 128        # embedding dim
F = 512        # input feature dim
EPS = 1e-5
GSZ = 512      # windows per chunk (one f32 PSUM bank)

# emission-schedule margins (batches after a gather's last x-write dep)
STATS_MARGIN = 99
GEMM_MARGIN = 5

_GRAPH_CACHE = {}
LAST_EXEC_NS = None
LAST_RESULT = None


def _bf16_dtype():
    import ml_dtypes

    return np.dtype(ml_dtypes.bfloat16)


def _fold_weights(W_enc, b_enc, gn_gamma, gn_beta, gn_alpha, cheb_W, cheb_b,
                  lin_W, lin_b):
    f32 = np.float32
    W0, W1, W2 = (np.asarray(cheb_W[i], f32) for i in range(3))
    gam = np.asarray(gn_gamma, f32)
    bet = np.asarray(gn_beta, f32)
    alp = np.asarray(gn_alpha, f32)
    Wa = W0 + W1 / f32(7.0) - f32(47.0 / 49.0) * W2
    Wb = -W1 / f32(7.0) + f32(12.0 / 49.0) * W2
    Wap = gam[:, None] * Wa
    Wd = ((f32(1.0) - alp) * gam)[:, None] * Wb
    D0 = bet @ Wa + f32(8.0) * (bet @ Wb) + np.asarray(cheb_b, f32)
    bf16 = _bf16_dtype()
    return {
        "wenc": np.ascontiguousarray(
            np.asarray(W_enc, f32).reshape(4, 128, 128).transpose(1, 0, 2)
            .astype(np.float16)),
        "benc": np.asarray(b_enc, f32).reshape(128, 1).copy(),
        "wap": np.ascontiguousarray(Wap.astype(bf16)),
        "wd": np.ascontiguousarray(Wd.astype(bf16)),
        "d0": np.ascontiguousarray(D0.reshape(128, 1)),
        "al8": np.ascontiguousarray((alp / f32(8.0)).reshape(128, 1)),
        "w12": np.ascontiguousarray(
            np.asarray(lin_W, f32).reshape(2, 128).T.astype(bf16)),  # [128, 2]
        "linb": np.asarray(lin_b, f32).reshape(1, 1).copy(),
        "ident": np.eye(128, dtype=bf16),
    }


def _build_graph(u_pad, e_pad, benc_zero=False, gdeps=None):
    """Build the per-core Bass graph. SPMD: same graph on all 8 cores."""
    import concourse.bass as bass
    import concourse.tile as tile
    from concourse import bacc, mybir

    f32 = mybir.dt.float32
    f32r = mybir.dt.float32r
    bf16 = mybir.dt.bfloat16
    f16 = mybir.dt.float16
    i16 = mybir.dt.int16
    AF = mybir.ActivationFunctionType
    OP = mybir.AluOpType

    nb = u_pad // 512          # encoder column tiles
    ng = e_pad // GSZ          # window chunks
    ns16 = e_pad // 16

    if gdeps is None:
        gdeps = tuple(nb for _ in range(ng))

    nc = bacc.Bacc()
    posT_p = nc.declare_dram_parameter("posT", [nb, 128, 4, 512], f16, False)
    idx_p = nc.declare_dram_parameter("idx", [128, ns16], i16, False)
    wenc_p = nc.declare_dram_parameter("wenc", [128, 4, 128], f16, False)
    benc_p = nc.declare_dram_parameter("benc", [128, 1], f32, False)
    wap_p = nc.declare_dram_parameter("wap", [128, 128], bf16, False)
    wd_p = nc.declare_dram_parameter("wd", [128, 128], bf16, False)
    d0_p = nc.declare_dram_parameter("d0", [128, 1], f32, False)
    al8_p = nc.declare_dram_parameter("al8", [128, 1], f32, False)
    w12_p = nc.declare_dram_parameter("w12", [128, 2], bf16, False)
    linb_p = nc.declare_dram_parameter("linb", [1, 1], f32, False)
    ident_p = nc.declare_dram_parameter("ident", [128, 128], bf16, False)
    out_p = nc.declare_dram_parameter("out", [1, e_pad], f32, True)

    # gather scratch: x rows, bf16, padded so overlapping window reads stay
    # in bounds
    x_dram = nc.dram_tensor("x_scratch", [u_pad + M, 128], bf16)

    # emission schedule: which chunks' stats blocks go after which encoder
    # batch (the rest, and all gemm+pool blocks, are emitted after the loop)
    stats_after = {}
    for c in range(ng):
        sb = gdeps[c] + STATS_MARGIN
        if sb <= nb - 1:
            stats_after.setdefault(sb - 1, []).append(c)

    with tile.TileContext(nc) as tc:
        with (
            tc.tile_pool(name="consts", bufs=1) as consts,
            tc.tile_pool(name="psum_enc", bufs=2, space="PSUM") as psum_enc,
            tc.tile_pool(name="psum_tr", bufs=1, space="PSUM") as psum_tr,
            tc.tile_pool(name="psum_gemm", bufs=2, space="PSUM") as psum_gemm,
            tc.tile_pool(name="psum_log", bufs=1, space="PSUM") as psum_log,
            tc.tile_pool(name="pos", bufs=3) as pos_pool,
            tc.tile_pool(name="xt", bufs=1) as xt_pool,
            tc.tile_pool(name="stage", bufs=2) as stage_pool,
            tc.tile_pool(name="xe", bufs=5) as xe_pool,
            tc.tile_pool(name="sq", bufs=2) as sq_pool,
            tc.tile_pool(name="xs", bufs=2) as xs_pool,
            tc.tile_pool(name="cu", bufs=2) as cu_pool,
            tc.tile_pool(name="stats", bufs=4) as stats_pool,
            tc.tile_pool(name="pools", bufs=3) as pools_pool,
            tc.tile_pool(name="tsc", bufs=3) as tree_pool,
            tc.tile_pool(name="fixed", bufs=1) as fixed_pool,
        ):
            # ---- constants ----
            wenc_t = consts.tile([128, 4, 128], f16)
            nc.scalar.dma_start(out=wenc_t[:, :, :], in_=wenc_p[:, :, :])
            benc_t = consts.tile([128, 1], f32)
            nc.scalar.dma_start(out=benc_t[:, :], in_=benc_p[:, :])
            wap_t = consts.tile([128, 128], bf16)
            nc.scalar.dma_start(out=wap_t[:, :], in_=wap_p[:, :])
            wd_t = consts.tile([128, 128], bf16)
            nc.scalar.dma_start(out=wd_t[:, :], in_=wd_p[:, :])
            d0_t = consts.tile([128, 1], f32)
            nc.scalar.dma_start(out=d0_t[:, :], in_=d0_p[:, :])
            al8_t = consts.tile([128, 1], f32)
            nc.scalar.dma_start(out=al8_t[:, :], in_=al8_p[:, :])
            w12_t = consts.tile([128, 2], bf16)
            nc.scalar.dma_start(out=w12_t[:, :], in_=w12_p[:, :])
            linb_t = consts.tile([1, 1], f32)
            nc.scalar.dma_start(out=linb_t[:, :], in_=linb_p[:, :])
            ident_t = consts.tile([128, 128], bf16)
            nc.scalar.dma_start(out=ident_t[:, :], in_=ident_p[:, :])
            idx_t = consts.tile([128, ns16], i16)
            nc.scalar.dma_start(out=idx_t[:, :], in_=idx_p[:, :])
            eps_t = consts.tile([128, 1], f32)
            nc.vector.memset(eps_t[:, :], EPS)
            from concourse import library_config
            nc.gpsimd.load_library(library_config.mlp)

            # pad rows of x_dram zeroed up front so gathers never wait on
            # the encoder tail for them
            zstg = fixed_pool.tile([M, 128], bf16, tag="zpad")
            nc.vector.memset(zstg[:, :], 0)
            padw = nc.gpsimd.dma_start(out=x_dram[u_pad:u_pad + M, :],
                                       in_=zstg[:, :])

            xT = xt_pool.tile([128, u_pad], bf16)  # [D, node]
            sig = fixed_pool.tile([1, e_pad], f32, tag="sig")

            x_writes = []
            gathers = []
            xes = [None] * ng
            st = [None] * ng     # per-chunk stats tiles

            x_view = bass.AP(tensor=x_dram, offset=0,
                             ap=[[128, u_pad], [1, M * 128]])
            nsc = GSZ // 16

            def vtree(dst, src, op, eng):
                # pairwise reduction over the 8 entry planes, all bf16,
                # one packed [128,512] op per tree edge (strided multi-plane
                # forms fall out of the DVE 2x fast path on hardware)
                t1 = tree_pool.tile([128, 4, GSZ], bf16, tag="tt")
                for j in range(4):
                    eng.tensor_tensor(
                        out=t1[:, j, :], in0=src[:, 2 * j, :],
                        in1=src[:, 2 * j + 1, :], op=op)
                eng.tensor_tensor(out=t1[:, 0, :], in0=t1[:, 0, :],
                                  in1=t1[:, 1, :], op=op)
                eng.tensor_tensor(out=t1[:, 2, :], in0=t1[:, 2, :],
                                  in1=t1[:, 3, :], op=op)
                eng.tensor_tensor(out=dst, in0=t1[:, 0, :],
                                  in1=t1[:, 2, :], op=op)

            def vtree_inplace(dst, src, op, eng_l1, eng_l23):
                # destructive tree: levels reuse the source planes (src is
                # dead after the reduction); level 1 may run on gpsimd
                eng_l1.tensor_tensor(out=src[:, 0, :], in0=src[:, 0, :],
                                     in1=src[:, 1, :], op=op)
                eng_l1.tensor_tensor(out=src[:, 1, :], in0=src[:, 2, :],
                                     in1=src[:, 3, :], op=op)
                eng_l1.tensor_tensor(out=src[:, 2, :], in0=src[:, 4, :],
                                     in1=src[:, 5, :], op=op)
                eng_l1.tensor_tensor(out=src[:, 3, :], in0=src[:, 6, :],
                                     in1=src[:, 7, :], op=op)
                eng_l23.tensor_tensor(out=src[:, 0, :], in0=src[:, 0, :],
                                      in1=src[:, 1, :], op=op)
                eng_l23.tensor_tensor(out=src[:, 2, :], in0=src[:, 2, :],
                                      in1=src[:, 3, :], op=op)
                eng_l23.tensor_tensor(out=dst, in0=src[:, 0, :],
                                      in1=src[:, 2, :], op=op)

            def emit_gather(c):
                xeT = xe_pool.tile([128, M, GSZ], bf16, tag="xe")
                xes[c] = xeT
                g = nc.gpsimd.dma_gather(
                    out_ap=xeT[:, :, :],
                    in_ap=x_view,
                    idxs_ap=idx_t[:, c * nsc:(c + 1) * nsc],
                    num_idxs=GSZ,
                    num_idxs_reg=GSZ,
                    elem_size=M * 128,
                    elem_step=128,
                    transpose=True,
                )
                for w in x_writes[:gdeps[c]]:
                    tile.add_dep_helper(g.ins, w.ins, reason="x_dram RAW")
                tile.add_dep_helper(g.ins, padw.ins, reason="x_dram pad RAW")
                gathers.append(g)

            def emit_stats(c):
                # window stats via CENTERED variance (ctr = x - alpha*mean):
                # sum-of-squares has no cancellation, so bf16 stays accurate
                # even for near-constant windows.  u_i folds to
                #   (r*ctr_i)@(g*Wa) + (r*S)@((1-a)g*Wb) + D0
                xeT = xes[c]
                S_bf = stats_pool.tile([128, GSZ], bf16, tag="S")
                am = stats_pool.tile([128, GSZ], bf16, tag="am")
                v = stats_pool.tile([128, GSZ], f32, tag="v")
                r = stats_pool.tile([128, GSZ], f32, tag="r")
                r_bf = stats_pool.tile([128, GSZ], bf16, tag="rb")
                p_bf = stats_pool.tile([128, GSZ], bf16, tag="p")
                sq = sq_pool.tile([128, M, GSZ], bf16, tag="sq")

                vtree(S_bf[:, :], xeT, OP.add, nc.vector)
                nc.vector.tensor_scalar(             # am = (alpha/8) * S
                    out=am[:, :], in0=S_bf[:, :], scalar1=al8_t[:, 0:1],
                    scalar2=None, op0=OP.mult)
                # ctr into the xs tile; later scaled by r in place
                xs = xs_pool.tile([128, M, GSZ], bf16, tag="xs")
                am_b = am[:, :].unsqueeze(1).broadcast_to((128, M, GSZ))
                nc.vector.tensor_tensor(out=xs[:, :, :], in0=xeT[:, :, :],
                                        in1=am_b, op=OP.subtract)
                # square in two halves so the ctr^2 tree level 1 starts
                # at half the ACT latency; tree fully on DVE (the gpsimd
                # level-1 variant adds two cross-engine hops and its ~1.5us
                # software adds sit on the stats critical chain)
                nc.scalar.activation(sq[:, 0:4, :], xs[:, 0:4, :], AF.Square)
                nc.scalar.activation(sq[:, 4:8, :], xs[:, 4:8, :], AF.Square)
                vtree(v[:, :], sq, OP.add, nc.vector)
                nc.scalar.activation(v[:, :], v[:, :], AF.Sqrt,
                                     bias=eps_t[:, 0:1],
                                     scale=0.125)    # sqrt(v/8+eps)
                nc.vector.reciprocal_approx_fast(out=r[:, :], in_=v[:, :])
                nc.vector.tensor_copy(out=r_bf[:, :], in_=r[:, :])
                nc.vector.tensor_tensor(out=p_bf[:, :], in0=r_bf[:, :],
                                        in1=S_bf[:, :], op=OP.mult)
                # xs = ctr * r (broadcast over the 8 entry planes, in place)
                r_b = r_bf[:, :].unsqueeze(1).broadcast_to((128, M, GSZ))
                nc.vector.tensor_tensor(out=xs[:, :, :], in0=xs[:, :, :],
                                        in1=r_b, op=OP.mult)
                st[c] = (p_bf, xs)

            def emit_gemm_pools(c):
                p_bf, xs = st[c]
                cs = slice(c * GSZ, (c + 1) * GSZ)
                cu = cu_pool.tile([128, M, GSZ], bf16, tag="cu")
                # per-entry GEMM, 2 entries per 2-bank PSUM tile with two
                # tiles in flight: the fused ACT identity+bias move of one
                # pair overlaps the matmuls of the next
                for t in range(M // 2):
                    ps2 = psum_gemm.tile([128, 2, GSZ], f32, tag="g")
                    for j in range(2):
                        nc.tensor.matmul(ps2[:, j, :], lhsT=wap_t[:, :],
                                         rhs=xs[:, 2 * t + j, :],
                                         start=True, stop=False)
                    for j in range(2):
                        nc.tensor.matmul(ps2[:, j, :], lhsT=wd_t[:, :],
                                         rhs=p_bf[:, :],
                                         start=False, stop=True)
                    nc.scalar.activation(cu[:, 2 * t:2 * t + 2, :],
                                         ps2[:, :, :], AF.Identity,
                                         bias=d0_t[:, 0:1], scale=1.0)
                # clip in two halves so the pool trees start at half the
                # clip latency
                nc.vector.tensor_scalar(
                    out=cu[:, 0:4, :], in0=cu[:, 0:4, :],
                    scalar1=1.0, scalar2=-1.0, op0=OP.min, op1=OP.max)
                nc.vector.tensor_scalar(
                    out=cu[:, 4:8, :], in0=cu[:, 4:8, :],
                    scalar1=1.0, scalar2=-1.0, op0=OP.min, op1=OP.max)
                umax = pools_pool.tile([128, GSZ], bf16, tag="ux")
                umin = pools_pool.tile([128, GSZ], bf16, tag="un")
                ymm = pools_pool.tile([128, GSZ], bf16, tag="ymm")
                ssq = pools_pool.tile([128, GSZ], bf16, tag="ssq")
                rms = pools_pool.tile([128, GSZ], bf16, tag="rms")
                vtree(umax[:, :], cu, OP.max, nc.vector)
                vtree(umin[:, :], cu, OP.min, nc.vector)
                nc.vector.tensor_tensor(out=ymm[:, :], in0=umax[:, :],
                                        in1=umin[:, :], op=OP.subtract)
                # rms pool: clip(u)^2 == min(u^2, 1); square in halves so
                # the sum tree starts at half the ACT latency
                sq2 = sq_pool.tile([128, M, GSZ], bf16, tag="sq")
                nc.scalar.activation(sq2[:, 0:4, :], cu[:, 0:4, :], AF.Square)
                nc.scalar.activation(sq2[:, 4:8, :], cu[:, 4:8, :], AF.Square)
                vtree(ssq[:, :], sq2, OP.add, nc.vector)
                nc.scalar.activation(rms[:, :], ssq[:, :], AF.Sqrt,
                                     scale=0.125)
                psl = psum_log.tile([1, GSZ], f32, tag="log")
                nc.tensor.matmul(psl[:, :], lhsT=w12_t[:, 0:1],
                                 rhs=ymm[:, :], start=True, stop=False)
                nc.tensor.matmul(psl[:, :], lhsT=w12_t[:, 1:2],
                                 rhs=rms[:, :], start=False, stop=True)
                nc.scalar.activation(sig[:, cs], psl[:, :], AF.Identity,
                                     bias=linb_t[0:1, 0:1], scale=1.0)

            # ---- encoder loop with interleaved phase-B emission ----
            for b in range(nb):
                pos_tile = pos_pool.tile([128, 4, 512], f16, tag="pos")
                nc.sync.dma_start(out=pos_tile[:, :, :], in_=posT_p[b, :, :, :])
                ps = psum_enc.tile([128, 512], f32, tag="enc")
                for k in range(4):
                    nc.tensor.matmul(
                        ps[:, :],
                        lhsT=wenc_t[:, k, :],
                        rhs=pos_tile[:, k, :],
                        start=(k == 0),
                        stop=(k == 3),
                    )
                bs = slice(b * 512, (b + 1) * 512)
                if benc_zero:
                    nc.vector.tensor_scalar(
                        out=xT[:, bs], in0=ps[:, :],
                        scalar1=1.0, scalar2=-1.0,
                        op0=OP.min, op1=OP.max)
                else:
                    nc.vector.tensor_scalar(
                        out=xT[:, bs], in0=ps[:, :],
                        scalar1=benc_t[:, 0:1], scalar2=1.0,
                        op0=OP.add, op1=OP.min)
                    nc.vector.tensor_scalar(
                        out=xT[:, bs], in0=xT[:, bs],
                        scalar1=-1.0, scalar2=None, op0=OP.max)
                pst = psum_tr.tile([128, 4, 128], bf16, tag="tr")
                for j in range(4):
                    t = 4 * b + j
                    nc.tensor.transpose(
                        out=pst[:, j, :],
                        in_=xT[:, t * 128:(t + 1) * 128],
                        identity=ident_t[:, :],
                    )
                stg = stage_pool.tile([128, 4, 128], bf16, tag="stage")
                nc.scalar.copy(out=stg[:, :, :], in_=pst[:, :, :])
                out_ap = bass.AP(
                    tensor=x_dram, offset=b * 512 * 128,
                    ap=[[128, 128], [128 * 128, 4], [1, 128]])
                w = nc.scalar.dma_start(out=out_ap, in_=stg[:, :, :])
                x_writes.append(w)
                # issue any gathers whose node range is now fully written
                for c in range(ng):
                    if gdeps[c] == b + 1 and xes[c] is None:
                        emit_gather(c)
            # leftover gathers (gdeps == nb), then phase B: stats one chunk
            # ahead of gemm+pools so the DVE never waits on PE/ACT
            for c in range(ng):
                if xes[c] is None:
                    emit_gather(c)
            emit_stats(0)
            for c in range(ng):
                if c + 1 < ng:
                    emit_stats(c + 1)
                emit_gemm_pools(c)

            # final sigmoid (one act-table switch), bulk emitted before
            # the last chunk's logits land so only [1, GSZ] sits on the
            # tail; bulk output DMA likewise overlaps the last chunk
            nc.scalar.activation(sig[:, 0:(ng - 1) * GSZ],
                                 sig[:, 0:(ng - 1) * GSZ], AF.Sigmoid)
            nc.sync.dma_start(out=out_p[:, 0:(ng - 1) * GSZ],
                              in_=sig[:, 0:(ng - 1) * GSZ])
            nc.scalar.activation(sig[:, (ng - 1) * GSZ:],
                                 sig[:, (ng - 1) * GSZ:], AF.Sigmoid)
            nc.sync.dma_start(out=out_p[:, (ng - 1) * GSZ:],
                              in_=sig[:, (ng - 1) * GSZ:])

    nc.finalize()
    return nc


def _np_fallback(pos_set, W_enc, b_enc, gn_gamma, gn_beta, gn_alpha, cheb_W,
                 cheb_b, lin_W, lin_b, members, edge_index, batch):
    """Pure-numpy general path (only used if the expected input structure is
    absent; inputs from setup_inputs always take the device path)."""
    f32 = np.float32
    E = members.shape[0]
    num_entries = members.size
    x = np.clip(pos_set @ W_enc + b_enc, -1.0, 1.0).astype(f32)
    xe = x[members.reshape(-1)]
    cnt = np.zeros(E, f32)
    np.add.at(cnt, batch, 1.0)
    mean = np.zeros((E, x.shape[1]), f32)
    np.add.at(mean, batch, xe)
    mean /= cnt[:, None]
    ctr = xe - gn_alpha * mean[batch]
    var = np.zeros((E, x.shape[1]), f32)
    np.add.at(var, batch, ctr * ctr)
    var /= cnt[:, None]
    xe = gn_gamma * ctr / np.sqrt(var + EPS)[batch] + gn_beta
    src, dst = edge_index[0], edge_index[1]
    deg = np.zeros(num_entries, f32)
    np.add.at(deg, dst, 1.0)
    w = -1.0 / np.sqrt(deg[src] * deg[dst])

    def lhat(h):
        out = np.zeros_like(h)
        np.add.at(out, dst, w[:, None] * h[src])
        return out

    tx0 = xe
    tx1 = lhat(tx0)
    out = tx0 @ cheb_W[0] + tx1 @ cheb_W[1]
    tkm1, tkm2 = tx1, tx0
    for k in range(2, cheb_W.shape[0]):
        tk = 2.0 * lhat(tkm1) - tkm2
        out = out + tk @ cheb_W[k]
        tkm1, tkm2 = tk, tkm1
    h = np.clip(out + cheb_b, -1.0, 1.0)
    ymax = np.full((E, h.shape[1]), -np.inf, f32)
    ymin = np.full((E, h.shape[1]), np.inf, f32)
    np.maximum.at(ymax, batch, h)
    np.minimum.at(ymin, batch, h)
    ynorm = np.zeros((E, h.shape[1]), f32)
    np.add.at(ynorm, batch, h * h)
    ynorm = np.sqrt(ynorm / cnt[:, None])
    y = np.concatenate([ymax - ymin, ynorm], axis=1)
    logits = y @ lin_W + lin_b
    return (1.0 / (1.0 + np.exp(-logits))).squeeze(-1).astype(f32)


def _has_window_structure(members, edge_index, batch):
    E, Mm = members.shape
    if Mm != M:
        return False
    base = members[:, 0]
    if not (members == base[:, None] + np.arange(M, dtype=members.dtype)).all():
        return False
    if not (batch == np.repeat(np.arange(E, dtype=batch.dtype), M)).all():
        return False
    row, col = np.where(~np.eye(M, dtype=bool))
    offs = np.arange(E, dtype=np.int64)[:, None] * M
    ei = np.stack([(offs + row[None, :]).ravel(), (offs + col[None, :]).ravel()])
    return (edge_index == ei).all()


def kernel(pos_set, W_enc, b_enc, gn_gamma, gn_beta, gn_alpha, cheb_W, cheb_b,
           lin_W, lin_b, members, edge_index, batch):
    pos_set = np.asarray(pos_set, np.float32)
    members = np.asarray(members)
    edge_index = np.asarray(edge_index)
    batch = np.asarray(batch)
    if not _has_window_structure(members, edge_index, batch):
        return _np_fallback(
            pos_set, np.asarray(W_enc, np.float32),
            np.asarray(b_enc, np.float32), np.asarray(gn_gamma, np.float32),
            np.asarray(gn_beta, np.float32), np.asarray(gn_alpha, np.float32),
            np.asarray(cheb_W, np.float32), np.asarray(cheb_b, np.float32),
            np.asarray(lin_W, np.float32), np.asarray(lin_b, np.float32),
            members, edge_index, batch)

    N = pos_set.shape[0]
    E = members.shape[0]
    base = members[:, 0].astype(np.int64)
    node_span = (N + N_CORES - 1) // N_CORES                # 6250
    u_pad = ((node_span + M + 511) // 512 + 1) * 512        # 6656 for N=50000
    # quantile split: sort windows by base, give each core an equal count.
    order = np.argsort(base, kind="stable")
    ecnt = (E + N_CORES - 1) // N_CORES
    counts = np.array([min(ecnt, E - c * ecnt) for c in range(N_CORES)])
    offs_pre = np.concatenate([[0], np.cumsum(counts)])
    node_lo = np.zeros(N_CORES, np.int64)
    ok = True
    for c in range(N_CORES):
        ids = order[offs_pre[c]:offs_pre[c + 1]]
        if ids.size == 0:
            node_lo[c] = 0
            continue
        node_lo[c] = base[ids[0]]
        if base[ids[-1]] + M - node_lo[c] > u_pad:
            ok = False
            break
    if not ok:
        core_of = np.minimum(base // node_span, N_CORES - 1)
        order = np.argsort(base, kind="stable")
        counts = np.bincount(core_of, minlength=N_CORES)
        offs_pre = np.concatenate([[0], np.cumsum(counts)])
        node_lo = np.arange(N_CORES, dtype=np.int64) * node_span
    e_pad = max(GSZ, int(math.ceil(counts.max() / GSZ)) * GSZ)

    benc_zero = bool(np.all(np.asarray(b_enc) == 0.0))
    ng_ = e_pad // GSZ
    nwb_ = u_pad // 512
    # per-chunk: how many 512-node x-write batches the gather depends on
    # (max over cores, from the actual window bases)
    gdeps = []
    for c in range(ng_):
        mx = 0
        for cc in range(N_CORES):
            ids = order[offs_pre[cc] + c * GSZ:
                        min(offs_pre[cc] + (c + 1) * GSZ, offs_pre[cc + 1])]
            if ids.size:
                mx = max(mx, int((base[ids] - node_lo[cc]).max()))
        gdeps.append(min(nwb_, (mx + M + 511) // 512))
    # hybrid gather schedule: only the first two chunks gather eagerly
    # (hides their stats chains under the encoder tail); the rest wait for
    # the full encoder so their transfers don't steal DMA bandwidth from
    # the pos loads that pace it
    gdeps = tuple(g if c < 3 else nwb_ for c, g in enumerate(gdeps))
    key = (u_pad, e_pad, benc_zero, gdeps)
    if key not in _GRAPH_CACHE:
        _GRAPH_CACHE[key] = _build_graph(u_pad, e_pad, benc_zero, gdeps)
    nc = _GRAPH_CACHE[key]

    shared = _fold_weights(W_enc, b_enc, gn_gamma, gn_beta, gn_alpha, cheb_W,
                           cheb_b, lin_W, lin_b)
    nb = u_pad // 512
    ns16 = e_pad // 16

    in_maps = []
    offs = offs_pre
    for c in range(N_CORES):
        lo = int(node_lo[c])
        sl = pos_set[lo:min(lo + u_pad, N)]
        if sl.shape[0] < u_pad:
            sl = np.concatenate(
                [sl, np.zeros((u_pad - sl.shape[0], F), np.float32)], 0)
        # posT[b, p, k, u'] = sl[512b+u', 128k+p]
        posT = np.ascontiguousarray(
            sl.reshape(nb, 512, 4, 128).transpose(0, 3, 2, 1)
            .astype(np.float16))
        ids = order[offs[c]:offs[c + 1]]
        loc = (base[ids] - lo).astype(np.int64)
        idx = np.zeros(e_pad, np.int16)
        idx[:loc.size] = loc.astype(np.int16)
        # wrapped layout: element i lives at [i % 16, i // 16], replicated
        # across the eight 16-partition groups
        w16 = idx.reshape(ns16, 16).T           # [16, ns16]
        m = dict(shared)
        m["posT"] = posT
        m["idx"] = np.ascontiguousarray(np.tile(w16, (8, 1)))
        in_maps.append(m)

    from concourse.bass_utils import run_bass_kernel_spmd

    trace = bool(os.environ.get("CHESHIRE_TRACE"))
    res = run_bass_kernel_spmd(nc, in_maps, core_ids=list(range(N_CORES)),
                               trace=trace)
    global LAST_EXEC_NS, LAST_RESULT
    LAST_EXEC_NS = res.exec_time_ns
    LAST_RESULT = res
    out_full = np.zeros(E, np.float32)
    for c in range(N_CORES):
        ids = order[offs[c]:offs[c + 1]]
        vals = np.asarray(res.results[c]["out"], np.float32).reshape(-1)
        out_full[ids] = vals[:ids.size]
    return out_full
